# revision 15
# baseline (speedup 1.0000x reference)
"""Trainium2 Bass kernel for nn_Attention_51376398794919.

Dense transformer block: LayerNorm -> QKV -> attention with relative-position
bias -> proj.  Data-parallel over batch across 8 NeuronCores (4 batches/core).

Device-side layout strategy (per core):
  - LN in natural layout [tok, d]; 1/sigma via a fast-inverse-sqrt bit trick
    + 2 Newton steps on the DVE (keeps the ACT engine on a single activation
    table set: only Exp and Copy are ever used -> one table load total);
    xn transposed to xnT [d, tok] via paired PE transposes (stored bf16).
  - qkT (q/k head-transposed, [d_head, tok]) computed from xnT and quantized
    to fp8e4m3 (scaled x2 on the host); scores run as a single fp8 DoubleRow
    matmul per (k-tile, chunk) with a stride-0 pair dim (reads each operand
    twice -> 2x result, folded into the exp scale 1/64).
  - Relative-position bias applied multiplicatively: host precomputes
    expb = exp(bias^T) bf16; est = exp(scores/64) (ACT) then est *= expb
    (DVE for k-tiles 0-4, GPSIMD for 5-7).  No bias matmuls on the PE.
  - PV: out[q, d] = expST.T @ [v | ones]; the ones column yields the softmax
    denominator; normalization on ACT (Copy with per-partition 1/den scale).
  - v-hat generated two token-slices per PSUM bank (shared-bank accumulation
    groups) so each PSUM->SBUF copy moves 512 columns; attn-out transposes
    are paired the same way.
Software pipelining: each head window interleaves, at k-tile granularity,
scores+exp+mult of head h+1, PV matmuls + normalization of head h, and the
deferred attn-out transposes/copies of head h-1 (which wait on the ACT
chain).  LN of batch b+1 is emitted during heads 4-6 of batch b; qkv/scores
of (head 0, b+1) run before pv(7, b); fin(7) interleaves into proj(b).
"""

import sys

import numpy as np

sys.path.insert(0, "/opt/trn_rl_repo")

import concourse.bacc as bacc
import concourse.mybir as mybir
import concourse.tile as tile
from concourse.bass_utils import run_bass_kernel_spmd

# Problem constants
B, N, DIM = 32, 1024, 512
H, KD, D = 8, 64, 256
DH = D * H  # 2048
SCALE = KD ** -0.5
NCORES = 8
BL = B // NCORES  # 4 batches per core

F32 = mybir.dt.float32
I32 = mybir.dt.int32
BF16 = mybir.dt.bfloat16
E4 = mybir.dt.float8e4
E5 = mybir.dt.float8e5
AF = mybir.ActivationFunctionType
ALU = mybir.AluOpType
DR = mybir.MatmulPerfMode.DoubleRow

KT = N // 128    # 8 k-tiles
QS = N // 128    # 8 q-slices
DT = DIM // 128  # 4 d-tiles
VW = 257         # v-hat width: 256 v + 1 ones
QK_PRE = 2.0     # extra pre-scale on q and k (on top of the x16 fp8 split)
W_SCALE = 16.0   # fp8 hi/lo weight split scale (wqk, wv)
# scores psum = 2 (stride-0 pair) * (F q)*(F k), F = 16*QK_PRE
EXP_SCALE = SCALE / (2 * (W_SCALE * QK_PRE) ** 2)
POOL_MULT_KT = (0, 1, 2)  # early k-tiles' est-mult on GPSIMD; late ones need the faster DVE


def dup2(ap):
    """Insert a stride-0 count-2 dim after the partition dim (DoubleRow
    pair that reads the same block twice -> result is 2x)."""
    new = ap.copy()
    new.ap = [ap.ap[0]] + [[0, 2]] + ap.ap[1:]
    return new


def build_program(use_qk_bias=False, use_v_bias=False, use_pb=False):
    nc = bacc.Bacc("TRN2", target_bir_lowering=False, debug=True)

    x_d = nc.declare_dram_parameter("x", [BL, N, DIM], F32, isOutput=False)
    wqkh_d = nc.declare_dram_parameter("wqkh", [DIM, H * 128], E4, isOutput=False)
    wqkl_d = nc.declare_dram_parameter("wqkl", [DIM, H * 128], E5, isOutput=False)
    wvh_d = nc.declare_dram_parameter("wvh", [DIM, DH], E4, isOutput=False)
    wvl_d = nc.declare_dram_parameter("wvl", [DIM, DH], E5, isOutput=False)
    bqk_d = nc.declare_dram_parameter("bqk", [1, H * 128], BF16, isOutput=False)
    bv1_d = nc.declare_dram_parameter("bv1", [1, H * VW], BF16, isOutput=False)
    pw_d = nc.declare_dram_parameter("pw", [DH, DIM], BF16, isOutput=False)
    pb1_d = nc.declare_dram_parameter("pb1", [1, DIM], BF16, isOutput=False)
    expb_d = nc.declare_dram_parameter("expb", [H, N, N], BF16, isOutput=False)
    identb_d = nc.declare_dram_parameter("identb", [128, 128], BF16, isOutput=False)
    ones_d = nc.declare_dram_parameter("ones", [1, 512], BF16, isOutput=False)
    y_d = nc.declare_dram_parameter("y", [BL, N, DIM], F32, isOutput=True)

    with tile.TileContext(nc) as tc:
        with (
            tc.tile_pool(name="consts", bufs=1) as cpool,
            tc.tile_pool(name="xnt", bufs=2) as xpool,
            tc.tile_pool(name="slab", bufs=1) as slabpool,
            tc.tile_pool(name="yout", bufs=3) as ypool,
            tc.tile_pool(name="lnx", bufs=8) as lxpool,
            tc.tile_pool(name="ln", bufs=3) as lpool,
            tc.tile_pool(name="stats", bufs=16) as spool,
            tc.tile_pool(name="bias", bufs=2) as bpool,
            tc.tile_pool(name="qk", bufs=3) as qkpool,
            tc.tile_pool(name="vhat", bufs=2) as vpool,
            tc.tile_pool(name="expst", bufs=16) as epool,
            tc.tile_pool(name="attn", bufs=10) as apool,
            tc.tile_pool(name="stp", bufs=2, space="PSUM") as stpp,
            tc.tile_pool(name="pvp", bufs=4, space="PSUM") as pvpp,
            tc.tile_pool(name="miscp", bufs=2, space="PSUM") as mpp,
        ):
            # ---- constants; identb first so LN transposes can start early
            identb = cpool.tile([128, 128], BF16)
            nc.sync.dma_start(identb[:], identb_d[:])
            eps_t = cpool.tile([128, 1], F32)
            nc.vector.memset(eps_t[:], 1e-5)

            def load_consts():
                if use_qk_bias or use_v_bias or use_pb:
                    ones_bf = cpool.tile([1, 512], BF16)
                    nc.sync.dma_start(ones_bf[:], ones_d[:])
                else:
                    ones_bf = None
                wqkh = cpool.tile([128, DT * H * 128], E4)
                wqkl = cpool.tile([128, DT * H * 128], E5)
                for dt in range(DT):
                    nc.sync.dma_start(
                        wqkh[:, dt * H * 128:(dt + 1) * H * 128],
                        wqkh_d[dt * 128:(dt + 1) * 128, :],
                    )
                    nc.sync.dma_start(
                        wqkl[:, dt * H * 128:(dt + 1) * H * 128],
                        wqkl_d[dt * 128:(dt + 1) * 128, :],
                    )
                if use_qk_bias:
                    bqk = cpool.tile([1, H * 128], BF16)
                    nc.sync.dma_start(bqk[:], bqk_d[:])
                else:
                    bqk = None
                wvh = cpool.tile([128, DT * DH], E4)
                wvl = cpool.tile([128, DT * DH], E5)
                for dt in range(DT):
                    nc.sync.dma_start(
                        wvh[:, dt * DH:(dt + 1) * DH],
                        wvh_d[dt * 128:(dt + 1) * 128, :],
                    )
                    nc.sync.dma_start(
                        wvl[:, dt * DH:(dt + 1) * DH],
                        wvl_d[dt * 128:(dt + 1) * 128, :],
                    )
                if use_v_bias:
                    bv1 = cpool.tile([1, H * VW], BF16)
                    nc.sync.dma_start(bv1[:], bv1_d[:])
                else:
                    bv1 = None
                pw = cpool.tile([128, 16 * DIM], BF16)
                for dh in range(16):
                    nc.sync.dma_start(
                        pw[:, dh * DIM:(dh + 1) * DIM],
                        pw_d[dh * 128:(dh + 1) * 128, :],
                    )
                if use_pb:
                    pb1 = cpool.tile([1, DIM], BF16)
                    nc.sync.dma_start(pb1[:], pb1_d[:])
                else:
                    pb1 = None
                return ones_bf, wqkh, wqkl, bqk, wvh, wvl, bv1, pw, pb1

            consts = [None]

            def _ln_stats(b, sl, j, sh):
                """DMA + bn stats for one LN slice; var into shared mvg."""
                xt = lxpool.tile([128, DIM], F32, tag="x", name=f"x{b}_{sl}")
                nc.sync.dma_start(xt[:], x_d[b, sl * 128:(sl + 1) * 128, :])
                sh["xts"][j] = xt
                st6 = spool.tile([128, 6], F32, tag="st6", name=f"s6{b}_{sl}")
                nc.vector.bn_stats(st6[:], xt[:])
                nc.vector.bn_aggr(sh["mvg"][:, 2 * j:2 * j + 2], st6[:])

            def _ln_chain(sh, L):
                """Batched fast-inverse-sqrt of var+eps over a group."""
                mvg = sh["mvg"]
                var_ap = mvg[:].rearrange("p (l two) -> p l two", two=2)[:, :, 1]
                ve = spool.tile([128, L], F32, tag="ve", name="ve")
                nc.vector.tensor_scalar(ve[:], var_ap, eps_t[:], None, ALU.add)
                ti = spool.tile([128, L], I32, tag="ti", name="ti")
                nc.vector.tensor_scalar(
                    ti[:], ve[:].bitcast(I32), 1, 0xFFFFFFFF,
                    ALU.logical_shift_right, ALU.bitwise_xor,
                )
                yi = spool.tile([128, L], I32, tag="yi", name="yi")
                nc.vector.tensor_scalar(yi[:], ti[:], 0x5F3759E0, None, ALU.add)
                y = yi[:].bitcast(F32)
                aa = spool.tile([128, L], F32, tag="aa", name="aa")
                cc = spool.tile([128, L], F32, tag="cc", name="cc")
                for _ in range(2):
                    nc.vector.tensor_tensor(aa[:], y, y, ALU.mult)
                    nc.vector.tensor_tensor(aa[:], aa[:], ve[:], ALU.mult)
                    nc.vector.tensor_scalar(
                        cc[:], aa[:], -0.5, 1.5, ALU.mult, ALU.add
                    )
                    nc.vector.tensor_tensor(y, y, cc[:], ALU.mult)
                sh["yi"] = yi

            def _ln_norm(b, sl, j, sh, xnth, xntl):
                """Normalize one slice and transpose into xnT hi/lo."""
                yi, mvg = sh["yi"], sh["mvg"]
                rs = yi[:, j:j + 1].bitcast(F32)
                nm = spool.tile([128, 1], F32, tag="nm", name=f"nm{b}_{sl}")
                nc.vector.tensor_scalar(
                    nm[:], mvg[:, 2 * j:2 * j + 1], rs, -1.0,
                    ALU.mult, ALU.mult
                )
                xn = lpool.tile([128, DIM], BF16, tag="xn", name=f"xn{b}_{sl}")
                nc.vector.tensor_scalar(
                    xn[:], sh["xts"][j][:], rs, nm[:], ALU.mult, ALU.add
                )
                for dp in range(2):  # pairs of d-tiles
                    tp = mpp.tile([128, 256], BF16, tag="m", name=f"tp{b}_{sl}")
                    for e in range(2):
                        nc.tensor.matmul(
                            tp[:, e * 128:(e + 1) * 128],
                            xn[:, (2 * dp + e) * 128:(2 * dp + e + 1) * 128],
                            identb[:], is_transpose=True,
                            start=(e == 0), stop=(e == 1),
                            skip_group_check=True,
                        )
                    hi_ap = (xnth[:].rearrange("p (d n) -> p d n", n=N)
                             [:, 2 * dp:2 * dp + 2, sl * 128:(sl + 1) * 128])
                    nc.vector.tensor_copy(
                        hi_ap, tp[:].rearrange("p (two n) -> p two n", two=2)
                    )
                    nc.vector.tensor_tensor(
                        xntl[:].rearrange("p (d n) -> p d n", n=N)
                            [:, 2 * dp:2 * dp + 2, sl * 128:(sl + 1) * 128],
                        tp[:].rearrange("p (two n) -> p two n", two=2),
                        hi_ap, ALU.subtract,
                    )

            def make_ln_tasks(b, xnth, xntl):
                """One closure per window cycle; groups of 4 slices."""
                tasks = []
                for g, grp in enumerate((range(0, 4), range(4, 8))):
                    L = len(grp)
                    sh = {"xts": [None] * L,
                          "mvg": spool.tile([128, 2 * L], F32, tag="mvg",
                                            name=f"mvg{b}_{g}")}
                    for j, sl in enumerate(grp):
                        tasks.append(lambda b=b, sl=sl, j=j, sh=sh:
                                     _ln_stats(b, sl, j, sh))
                    def chain_and_first(b=b, grp=grp, sh=sh, L=L):
                        _ln_chain(sh, L)
                        _ln_norm(b, grp[0], 0, sh, xnth, xntl)
                    tasks.append(chain_and_first)
                    for j, sl in enumerate(grp):
                        if j == 0:
                            continue
                        tasks.append(lambda b=b, sl=sl, j=j, sh=sh:
                                     _ln_norm(b, sl, j, sh, xnth, xntl))
                return tasks

            def emit_ln(b, xnth, xntl):
                for t in make_ln_tasks(b, xnth, xntl):
                    t()

            def emit_qkv_qk(h, xnth, xntl):
                """expb tile, qT/kT (fp8) for head h; 3-term fp8 DoubleRow
                (xh@wh + xl@wh + xh@wl).  v-hat is allocated here but its
                pairs are emitted inside the window interleave."""
                ones_bf, wqkh, wqkl, bqk, wvh, wvl, bv1, pw, pb1 = consts[0]
                bt = bpool.tile([128, KT * N], BF16, tag="bias")
                nc.sync.dma_start(
                    bt[:].rearrange("p (k n) -> p k n", n=N),
                    expb_d[h].rearrange("(k p) n -> p k n", p=128),
                )
                qt = qkpool.tile([64, N], E4, tag="qt")
                ktile = qkpool.tile([64, N], E4, tag="kt")
                wq_ap = lambda w, p: (
                    w[:].rearrange("p (d c) -> p d c", c=H * 128)
                    [:, 2 * p:2 * p + 2, h * 128:(h + 1) * 128])
                x_ap = lambda x, p, lo, width: (
                    x[:].rearrange("p (d n) -> p d n", n=N)
                    [:, 2 * p:2 * p + 2, lo:lo + width])
                for c in range(2):
                    qp = mpp.tile([128, 512], F32, tag="m")
                    terms = [(wqkh, xnth), (wqkh, xntl), (wqkl, xnth)]
                    for t, (w, x) in enumerate(terms):
                        for p in range(2):
                            nc.tensor.matmul(
                                qp[:], wq_ap(w, p), x_ap(x, p, c * 512, 512),
                                start=(t == 0 and p == 0),
                                stop=(not use_qk_bias and t == 2 and p == 1),
                                perf_mode=DR, skip_group_check=True,
                            )
                    if use_qk_bias:
                        nc.tensor.matmul(
                            qp[:],
                            bqk[:, h * 128:(h + 1) * 128],
                            ones_bf[:, 0:512],
                            start=False,
                            stop=True,
                        )
                    nc.vector.tensor_copy(
                        qt[:, c * 512:(c + 1) * 512], qp[0:64, :]
                    )
                    nc.vector.tensor_copy(
                        ktile[:, c * 512:(c + 1) * 512], qp[64:128, :]
                    )
                vh = vpool.tile([128, KT * VW], BF16, tag="vh")
                nc.vector.memset(
                    vh[:].rearrange("p (s w) -> p s w", w=VW)[:, :, 256:257],
                    1.0,
                )
                return (h, bt, qt, ktile, vh, xnth, xntl)

            def emit_qkv_vpair(hctx, sp):
                """One PSUM bank worth of v-hat (two tok-slices)."""
                h, bt, qt, ktile, vh, xnth, xntl = hctx
                ones_bf, wqkh, wqkl, bqk, wvh, wvl, bv1, pw, pb1 = consts[0]
                x_ap = lambda x, p, lo, width: (
                    x[:].rearrange("p (d n) -> p d n", n=N)
                    [:, 2 * p:2 * p + 2, lo:lo + width])
                wv_ap = lambda w, p: (
                    w[:].rearrange("p (d z) -> p d z", z=DH)
                    [:, 2 * p:2 * p + 2, h * 256:(h + 1) * 256])
                vp = pvpp.tile([128, 512], F32, tag="pv")
                for e in range(2):
                    sl = 2 * sp + e
                    last = (e == 1 and not use_v_bias)
                    terms = [(xnth, wvh), (xntl, wvh), (xnth, wvl)]
                    for t, (x, w) in enumerate(terms):
                        for p in range(2):
                            nc.tensor.matmul(
                                vp[:, e * 256:(e + 1) * 256],
                                x_ap(x, p, sl * 128, 128), wv_ap(w, p),
                                start=(e == 0 and t == 0 and p == 0),
                                stop=(last and t == 2 and p == 1),
                                perf_mode=DR, skip_group_check=True,
                            )
                    if use_v_bias:
                        nc.tensor.matmul(
                            vp[:, e * 256:(e + 1) * 256],
                            ones_bf[:, 0:128],
                            bv1[:, h * VW: h * VW + 256],
                            start=False,
                            stop=(e == 1),
                            skip_group_check=True,
                        )
                nc.vector.tensor_scalar(
                    vh[:].rearrange("p (s w) -> p s w", w=VW)
                       [:, 2 * sp:2 * sp + 2, 0:256],
                    vp[:].rearrange("p (two v) -> p two v", two=2),
                    1.0 / W_SCALE, None, ALU.mult,
                )

            def emit_st_kt(hctx, kt):
                """Scores DR matmul + exp + expb-mult for one k-tile of the
                *next* head.  Returns the est tile."""
                h, bt, qt, ktile, vh, xnth, xntl = hctx
                et = epool.tile([128, N], BF16, tag="e")
                for c in range(2):
                    cs = slice(c * 512, (c + 1) * 512)
                    sp = stpp.tile([128, 512], F32, tag="st")
                    nc.tensor.matmul(
                        sp[:],
                        dup2(ktile[:, kt * 128:(kt + 1) * 128]),
                        dup2(qt[:, cs]),
                        start=True, stop=True, perf_mode=DR,
                    )
                    nc.scalar.activation(
                        et[:, cs], sp[:], AF.Exp, bias=0.0, scale=EXP_SCALE
                    )
                eng = nc.gpsimd if kt in POOL_MULT_KT else nc.vector
                eng.tensor_tensor(
                    et[:], et[:], bt[:, kt * N:(kt + 1) * N], ALU.mult
                )
                return et

            def emit_pv_sl(hctx, est, sl):
                """PV matmuls + denominator + normalized attn for one
                tok-slice.  Returns the an tile."""
                h, bt, qt, ktile, vh, xnth, xntl = hctx
                pv = pvpp.tile([128, VW], F32, tag="pv")
                for kt in range(KT):
                    nc.tensor.matmul(
                        pv[:],
                        est[kt][:, sl * 128:(sl + 1) * 128],
                        vh[:, kt * VW:(kt + 1) * VW],
                        start=(kt == 0),
                        stop=(kt == KT - 1),
                    )
                rc = spool.tile([128, 1], F32, tag="rc")
                nc.vector.reciprocal(rc[:], pv[:, 256:257])
                an = apool.tile([128, 256], BF16, tag="an")
                nc.scalar.mul(an[:], pv[:, 0:256], rc[:])
                return an

            def emit_fin_sl(h, ans, sl, slab):
                """Deferred: paired transpose of normalized attn into slab."""
                tp = mpp.tile([128, 256], BF16, tag="m")
                for e in range(2):
                    nc.tensor.matmul(
                        tp[:, e * 128:(e + 1) * 128],
                        ans[sl][:, e * 128:(e + 1) * 128],
                        identb[:], is_transpose=True,
                        start=(e == 0), stop=(e == 1),
                        skip_group_check=True,
                    )
                nc.vector.tensor_copy(
                    slab[:].rearrange("p (g n) -> p g n", n=N)
                        [:, 2 * h:2 * h + 2, sl * 128:(sl + 1) * 128],
                    tp[:].rearrange("p (two n) -> p two n", two=2),
                )

            def emit_window(hctx_cur, est_cur, hctx_nxt, pend, slab, ln_q):
                """One head window: interleave scores/exp/mult of the next
                head, PV of the current head, fin of the previous, and at
                most one LN sub-task per cycle."""
                est_nxt = [] if hctx_nxt is not None else None
                ans = []
                for i in range(KT):
                    if hctx_nxt is not None:
                        est_nxt.append(emit_st_kt(hctx_nxt, i))
                        if i % 2 == 1:
                            emit_qkv_vpair(hctx_nxt, i // 2)
                    ans.append(emit_pv_sl(hctx_cur, est_cur, i))
                    if pend is not None:
                        emit_fin_sl(pend[0], pend[1], i, slab)
                    if ln_q:
                        ln_q.pop(0)()
                return est_nxt, ans

            def emit_proj(b, slab, pend):
                ones_bf, wqkh, wqkl, bqk, wvh, wvl, bv1, pw, pb1 = consts[0]
                for sl in range(QS):
                    if pend is not None:
                        emit_fin_sl(pend[0], pend[1], sl, slab)
                    pp = stpp.tile([128, DIM], F32, tag="st")
                    for dh in range(16):
                        nc.tensor.matmul(
                            pp[:],
                            slab[:, dh * N + sl * 128: dh * N + (sl + 1) * 128],
                            pw[:, dh * DIM:(dh + 1) * DIM],
                            start=(dh == 0),
                            stop=(not use_pb and dh == 15),
                        )
                    if use_pb:
                        nc.tensor.matmul(
                            pp[:], ones_bf[:, 0:128], pb1[:], start=False,
                            stop=True, skip_group_check=True,
                        )
                    yt = ypool.tile([128, DIM], F32, tag="y")
                    nc.vector.tensor_copy(yt[:], pp[:])
                    nc.sync.dma_start(y_d[b, sl * 128:(sl + 1) * 128, :], yt[:])

            # ---- software-pipelined main loop --------------------------
            xnt_cur = (xpool.tile([128, DT * N], E4, tag="xnth", name="xnth1"),
                       xpool.tile([128, DT * N], E5, tag="xntl", name="xntl1"))
            emit_ln(0, *xnt_cur)
            consts[0] = load_consts()
            hctx_cur = emit_qkv_qk(0, *xnt_cur)
            for sp in range(QS // 2):
                emit_qkv_vpair(hctx_cur, sp)
            est_cur = [emit_st_kt(hctx_cur, kt) for kt in range(KT)]
            slab = slabpool.tile([128, 16 * N], BF16, tag="slab")
            xnt_next = None
            pend = None
            ln_q = []
            for b in range(BL):
                for h in range(H):
                    if h + 1 < H:
                        hctx_nxt = emit_qkv_qk(h + 1, *xnt_cur)
                        if b + 1 < BL and h == 3:
                            xnt_next = (
                                xpool.tile([128, DT * N], E4,
                                           tag="xnth", name="xnth2"),
                                xpool.tile([128, DT * N], E5,
                                           tag="xntl", name="xntl2"),
                            )
                            ln_q.extend(make_ln_tasks(b + 1, *xnt_next))
                    elif b + 1 < BL:
                        hctx_nxt = emit_qkv_qk(0, *xnt_next)
                    else:
                        hctx_nxt = None
                    est_nxt, ans = emit_window(
                        hctx_cur, est_cur, hctx_nxt, pend, slab, ln_q
                    )
                    pend = (h, ans)
                    hctx_cur, est_cur = hctx_nxt, est_nxt
                emit_proj(b, slab, pend)
                pend = None
                xnt_cur = xnt_next

    nc.compile()
    return nc


_CACHE = {}


def _prep_host(gamma, beta, qkv_w, qkv_b, proj_w, proj_b, biases, bias_idxs):
    import ml_dtypes

    qkv_w = np.asarray(qkv_w, np.float32)
    qkv_b = np.asarray(qkv_b, np.float32)
    gamma = np.asarray(gamma, np.float32)
    beta = np.asarray(beta, np.float32)
    w = qkv_w * gamma[:, None]          # fold LN gamma
    bfold = qkv_b + beta @ qkv_w        # fold LN beta
    w3 = w.reshape(DIM, H, 384)
    b3 = bfold.reshape(H, 384)
    # q/k columns scaled x(16*QK_PRE); v columns x16; exp scale / the 1/16
    # copy-out divide it back.  Weights split hi (e4m3) + lo (e5m2).
    wqk = (w3[:, :, :128] * (QK_PRE * W_SCALE)).reshape(DIM, H * 128)
    bqk = (b3[:, :128] * (QK_PRE * W_SCALE)).reshape(1, H * 128)
    wv = (w3[:, :, 128:] * W_SCALE).reshape(DIM, DH)
    bv = b3[:, 128:] * W_SCALE          # [H, 256]
    bv1 = np.concatenate(
        [bv, np.ones((H, 1), np.float32)], axis=1,
    ).reshape(1, H * VW)
    wqkh = wqk.astype(ml_dtypes.float8_e4m3)
    wqkl = (wqk - wqkh.astype(np.float32)).astype(ml_dtypes.float8_e5m2)
    wvh = wv.astype(ml_dtypes.float8_e4m3)
    wvl = (wv - wvh.astype(np.float32)).astype(ml_dtypes.float8_e5m2)
    bias_full = np.asarray(biases, np.float32)[:, np.asarray(bias_idxs)]
    # device multiplies est[k, q] by exp(bias[q, k])^T
    expb = np.exp(bias_full.transpose(0, 2, 1))
    return {
        "wqkh": wqkh,
        "wqkl": wqkl,
        "wvh": wvh,
        "wvl": wvl,
        "bqk": bqk.astype(ml_dtypes.bfloat16),
        "bv1": bv1.astype(ml_dtypes.bfloat16),
        "pw": np.ascontiguousarray(np.asarray(proj_w, np.float32)).astype(ml_dtypes.bfloat16),
        "pb1": np.asarray(proj_b, np.float32).reshape(1, DIM).astype(ml_dtypes.bfloat16),
        "expb": np.ascontiguousarray(expb).astype(ml_dtypes.bfloat16),
        "identb": np.eye(128, dtype=np.float32).astype(ml_dtypes.bfloat16),
        "ones": np.ones((1, 512), ml_dtypes.bfloat16),
    }


def kernel(x, gamma, beta, qkv_w, qkv_b, proj_w, proj_b, biases, bias_idxs,
           _trace=False, _tmpdir=None):
    x = np.asarray(x, np.float32)
    shared = _prep_host(gamma, beta, qkv_w, qkv_b, proj_w, proj_b, biases,
                        bias_idxs)
    flags = (
        bool(np.any(np.asarray(shared["bqk"], np.float32))),
        bool(np.any(np.asarray(shared["bv1"], np.float32)
                    .reshape(H, VW)[:, :256])),
        bool(np.any(np.asarray(shared["pb1"], np.float32))),
    )
    if _CACHE.get("flags") != flags:
        _CACHE["nc"] = build_program(*flags)
        _CACHE["flags"] = flags
    nc = _CACHE["nc"]
    in_maps = []
    for c in range(NCORES):
        m = dict(shared)
        m["x"] = np.ascontiguousarray(x[c * BL:(c + 1) * BL])
        in_maps.append(m)
    res = run_bass_kernel_spmd(
        nc, in_maps, list(range(NCORES)), trace=_trace, tmpdir=_tmpdir,
    )
    _CACHE["last"] = res
    out = np.concatenate([res.results[c]["y"] for c in range(NCORES)], axis=0)
    return out.astype(np.float32)


# revision 16
# speedup vs baseline: 1.0133x; 1.0133x over previous
"""Trainium2 Bass kernel for nn_Attention_51376398794919.

Dense transformer block: LayerNorm -> QKV -> attention with relative-position
bias -> proj.  Data-parallel over batch across 8 NeuronCores (4 batches/core).

Device-side layout strategy (per core):
  - LN in natural layout [tok, d]; 1/sigma via a fast-inverse-sqrt bit trick
    + 2 Newton steps on the DVE (keeps the ACT engine on a single activation
    table set: only Exp and Copy are ever used -> one table load total);
    xn transposed to xnT [d, tok] via paired PE transposes (stored bf16).
  - qkT (q/k head-transposed, [d_head, tok]) computed from xnT and quantized
    to fp8e4m3 (scaled x2 on the host); scores run as a single fp8 DoubleRow
    matmul per (k-tile, chunk) with a stride-0 pair dim (reads each operand
    twice -> 2x result, folded into the exp scale 1/64).
  - Relative-position bias applied multiplicatively: host precomputes
    expb = exp(bias^T) bf16; est = exp(scores/64) (ACT) then est *= expb
    (DVE for k-tiles 0-4, GPSIMD for 5-7).  No bias matmuls on the PE.
  - PV: out[q, d] = expST.T @ [v | ones]; the ones column yields the softmax
    denominator; normalization on ACT (Copy with per-partition 1/den scale).
  - v-hat generated two token-slices per PSUM bank (shared-bank accumulation
    groups) so each PSUM->SBUF copy moves 512 columns; attn-out transposes
    are paired the same way.
Software pipelining: each head window interleaves, at k-tile granularity,
scores+exp+mult of head h+1, PV matmuls + normalization of head h, and the
deferred attn-out transposes/copies of head h-1 (which wait on the ACT
chain).  LN of batch b+1 is emitted during heads 4-6 of batch b; qkv/scores
of (head 0, b+1) run before pv(7, b); fin(7) interleaves into proj(b).
"""

import sys

import numpy as np

sys.path.insert(0, "/opt/trn_rl_repo")

import concourse.bacc as bacc
import concourse.mybir as mybir
import concourse.tile as tile
from concourse.bass_utils import run_bass_kernel_spmd

# Problem constants
B, N, DIM = 32, 1024, 512
H, KD, D = 8, 64, 256
DH = D * H  # 2048
SCALE = KD ** -0.5
NCORES = 8
BL = B // NCORES  # 4 batches per core

F32 = mybir.dt.float32
I32 = mybir.dt.int32
BF16 = mybir.dt.bfloat16
E4 = mybir.dt.float8e4
E5 = mybir.dt.float8e5
AF = mybir.ActivationFunctionType
ALU = mybir.AluOpType
DR = mybir.MatmulPerfMode.DoubleRow

KT = N // 128    # 8 k-tiles
QS = N // 128    # 8 q-slices
DT = DIM // 128  # 4 d-tiles
VW = 257         # v-hat width: 256 v + 1 ones
QK_PRE = 2.0     # extra pre-scale on q and k (on top of the x16 fp8 split)
W_SCALE = 16.0   # fp8 hi/lo weight split scale (wqk, wv)
# scores psum = 2 (stride-0 pair) * (F q)*(F k), F = 16*QK_PRE
EXP_SCALE = SCALE / (2 * (W_SCALE * QK_PRE) ** 2)
POOL_MULT_KT = (0, 1, 2)  # early k-tiles' est-mult on GPSIMD; late ones need the faster DVE


def dup2(ap):
    """Insert a stride-0 count-2 dim after the partition dim (DoubleRow
    pair that reads the same block twice -> result is 2x)."""
    new = ap.copy()
    new.ap = [ap.ap[0]] + [[0, 2]] + ap.ap[1:]
    return new


def build_program(use_qk_bias=False, use_v_bias=False, use_pb=False):
    nc = bacc.Bacc("TRN2", target_bir_lowering=False, debug=True)

    x_d = nc.declare_dram_parameter("x", [BL, N, DIM], F32, isOutput=False)
    wqkh_d = nc.declare_dram_parameter("wqkh", [DIM, H * 128], E4, isOutput=False)
    wqkl_d = nc.declare_dram_parameter("wqkl", [DIM, H * 128], E5, isOutput=False)
    wvh_d = nc.declare_dram_parameter("wvh", [DIM, DH], E4, isOutput=False)
    wvl_d = nc.declare_dram_parameter("wvl", [DIM, DH], E5, isOutput=False)
    bqk_d = nc.declare_dram_parameter("bqk", [1, H * 128], BF16, isOutput=False)
    bv1_d = nc.declare_dram_parameter("bv1", [1, H * VW], BF16, isOutput=False)
    pw_d = nc.declare_dram_parameter("pw", [DH, DIM], BF16, isOutput=False)
    pb1_d = nc.declare_dram_parameter("pb1", [1, DIM], BF16, isOutput=False)
    expb_d = nc.declare_dram_parameter("expb", [H, N, N], BF16, isOutput=False)
    identb_d = nc.declare_dram_parameter("identb", [128, 128], BF16, isOutput=False)
    ones_d = nc.declare_dram_parameter("ones", [1, 512], BF16, isOutput=False)
    y_d = nc.declare_dram_parameter("y", [BL, N, DIM], F32, isOutput=True)

    with tile.TileContext(nc) as tc:
        with (
            tc.tile_pool(name="consts", bufs=1) as cpool,
            tc.tile_pool(name="xnt", bufs=2) as xpool,
            tc.tile_pool(name="slab", bufs=1) as slabpool,
            tc.tile_pool(name="yout", bufs=3) as ypool,
            tc.tile_pool(name="lnx", bufs=8) as lxpool,
            tc.tile_pool(name="ln", bufs=3) as lpool,
            tc.tile_pool(name="stats", bufs=16) as spool,
            tc.tile_pool(name="bias", bufs=2) as bpool,
            tc.tile_pool(name="qk", bufs=3) as qkpool,
            tc.tile_pool(name="vhat", bufs=2) as vpool,
            tc.tile_pool(name="expst", bufs=16) as epool,
            tc.tile_pool(name="attn", bufs=10) as apool,
            tc.tile_pool(name="stp", bufs=2, space="PSUM") as stpp,
            tc.tile_pool(name="pvp", bufs=4, space="PSUM") as pvpp,
            tc.tile_pool(name="miscp", bufs=2, space="PSUM") as mpp,
        ):
            # ---- constants; identb first so LN transposes can start early
            identb = cpool.tile([128, 128], BF16)
            nc.sync.dma_start(identb[:], identb_d[:])
            eps_t = cpool.tile([128, 1], F32)
            nc.vector.memset(eps_t[:], 1e-5)

            def load_consts():
                if use_qk_bias or use_v_bias or use_pb:
                    ones_bf = cpool.tile([1, 512], BF16)
                    nc.sync.dma_start(ones_bf[:], ones_d[:])
                else:
                    ones_bf = None
                wqkh = cpool.tile([128, DT * H * 128], E4)
                wqkl = cpool.tile([128, DT * H * 128], E5)
                for dt in range(DT):
                    nc.sync.dma_start(
                        wqkh[:, dt * H * 128:(dt + 1) * H * 128],
                        wqkh_d[dt * 128:(dt + 1) * 128, :],
                    )
                    nc.sync.dma_start(
                        wqkl[:, dt * H * 128:(dt + 1) * H * 128],
                        wqkl_d[dt * 128:(dt + 1) * 128, :],
                    )
                if use_qk_bias:
                    bqk = cpool.tile([1, H * 128], BF16)
                    nc.sync.dma_start(bqk[:], bqk_d[:])
                else:
                    bqk = None
                wvh = cpool.tile([128, DT * DH], E4)
                wvl = cpool.tile([128, DT * DH], E5)
                for dt in range(DT):
                    nc.sync.dma_start(
                        wvh[:, dt * DH:(dt + 1) * DH],
                        wvh_d[dt * 128:(dt + 1) * 128, :],
                    )
                    nc.sync.dma_start(
                        wvl[:, dt * DH:(dt + 1) * DH],
                        wvl_d[dt * 128:(dt + 1) * 128, :],
                    )
                if use_v_bias:
                    bv1 = cpool.tile([1, H * VW], BF16)
                    nc.sync.dma_start(bv1[:], bv1_d[:])
                else:
                    bv1 = None
                if use_pb:
                    pb1 = cpool.tile([1, DIM], BF16)
                    nc.sync.dma_start(pb1[:], pb1_d[:])
                else:
                    pb1 = None
                pw = cpool.tile([128, 16 * DIM], BF16)
                return ones_bf, wqkh, wqkl, bqk, wvh, wvl, bv1, pw, pb1

            def load_pw():
                pw = consts[0][7]
                for dh in range(16):
                    nc.sync.dma_start(
                        pw[:, dh * DIM:(dh + 1) * DIM],
                        pw_d[dh * 128:(dh + 1) * 128, :],
                    )

            consts = [None]

            def _ln_stats(b, sl, j, sh):
                """DMA + bn stats for one LN slice; var into shared mvg."""
                xt = lxpool.tile([128, DIM], F32, tag="x", name=f"x{b}_{sl}")
                nc.sync.dma_start(xt[:], x_d[b, sl * 128:(sl + 1) * 128, :])
                sh["xts"][j] = xt
                st6 = spool.tile([128, 6], F32, tag="st6", name=f"s6{b}_{sl}")
                nc.vector.bn_stats(st6[:], xt[:])
                nc.vector.bn_aggr(sh["mvg"][:, 2 * j:2 * j + 2], st6[:])

            def _ln_chain(sh, L):
                """Batched fast-inverse-sqrt of var+eps over a group."""
                mvg = sh["mvg"]
                var_ap = mvg[:].rearrange("p (l two) -> p l two", two=2)[:, :, 1]
                ve = spool.tile([128, L], F32, tag="ve", name="ve")
                nc.vector.tensor_scalar(ve[:], var_ap, eps_t[:], None, ALU.add)
                ti = spool.tile([128, L], I32, tag="ti", name="ti")
                nc.vector.tensor_scalar(
                    ti[:], ve[:].bitcast(I32), 1, 0xFFFFFFFF,
                    ALU.logical_shift_right, ALU.bitwise_xor,
                )
                yi = spool.tile([128, L], I32, tag="yi", name="yi")
                nc.vector.tensor_scalar(yi[:], ti[:], 0x5F3759E0, None, ALU.add)
                y = yi[:].bitcast(F32)
                aa = spool.tile([128, L], F32, tag="aa", name="aa")
                cc = spool.tile([128, L], F32, tag="cc", name="cc")
                for _ in range(2):
                    nc.vector.tensor_tensor(aa[:], y, y, ALU.mult)
                    nc.vector.tensor_tensor(aa[:], aa[:], ve[:], ALU.mult)
                    nc.vector.tensor_scalar(
                        cc[:], aa[:], -0.5, 1.5, ALU.mult, ALU.add
                    )
                    nc.vector.tensor_tensor(y, y, cc[:], ALU.mult)
                sh["yi"] = yi

            def _ln_norm(b, sl, j, sh, xnth, xntl):
                """Normalize one slice and transpose into xnT hi/lo."""
                yi, mvg = sh["yi"], sh["mvg"]
                rs = yi[:, j:j + 1].bitcast(F32)
                nm = spool.tile([128, 1], F32, tag="nm", name=f"nm{b}_{sl}")
                nc.vector.tensor_scalar(
                    nm[:], mvg[:, 2 * j:2 * j + 1], rs, -1.0,
                    ALU.mult, ALU.mult
                )
                xn = lpool.tile([128, DIM], BF16, tag="xn", name=f"xn{b}_{sl}")
                nc.vector.tensor_scalar(
                    xn[:], sh["xts"][j][:], rs, nm[:], ALU.mult, ALU.add
                )
                for dp in range(2):  # pairs of d-tiles
                    tp = mpp.tile([128, 256], BF16, tag="m", name=f"tp{b}_{sl}")
                    for e in range(2):
                        nc.tensor.matmul(
                            tp[:, e * 128:(e + 1) * 128],
                            xn[:, (2 * dp + e) * 128:(2 * dp + e + 1) * 128],
                            identb[:], is_transpose=True,
                            start=(e == 0), stop=(e == 1),
                            skip_group_check=True,
                        )
                    hi_ap = (xnth[:].rearrange("p (d n) -> p d n", n=N)
                             [:, 2 * dp:2 * dp + 2, sl * 128:(sl + 1) * 128])
                    nc.vector.tensor_copy(
                        hi_ap, tp[:].rearrange("p (two n) -> p two n", two=2)
                    )
                    nc.vector.tensor_tensor(
                        xntl[:].rearrange("p (d n) -> p d n", n=N)
                            [:, 2 * dp:2 * dp + 2, sl * 128:(sl + 1) * 128],
                        tp[:].rearrange("p (two n) -> p two n", two=2),
                        hi_ap, ALU.subtract,
                    )

            def make_ln_tasks(b, xnth, xntl):
                """One closure per window cycle; groups of 4 slices."""
                tasks = []
                for g, grp in enumerate((range(0, 4), range(4, 8))):
                    L = len(grp)
                    sh = {"xts": [None] * L,
                          "mvg": spool.tile([128, 2 * L], F32, tag="mvg",
                                            name=f"mvg{b}_{g}")}
                    for j, sl in enumerate(grp):
                        tasks.append(lambda b=b, sl=sl, j=j, sh=sh:
                                     _ln_stats(b, sl, j, sh))
                    def chain_and_first(b=b, grp=grp, sh=sh, L=L):
                        _ln_chain(sh, L)
                        _ln_norm(b, grp[0], 0, sh, xnth, xntl)
                    tasks.append(chain_and_first)
                    for j, sl in enumerate(grp):
                        if j == 0:
                            continue
                        tasks.append(lambda b=b, sl=sl, j=j, sh=sh:
                                     _ln_norm(b, sl, j, sh, xnth, xntl))
                return tasks

            def emit_ln_group(b, grp, g, xnth, xntl):
                L = len(grp)
                sh = {"xts": [None] * L,
                      "mvg": spool.tile([128, 2 * L], F32, tag="mvg",
                                        name=f"mvg{b}_{g}")}
                for j, sl in enumerate(grp):
                    _ln_stats(b, sl, j, sh)
                _ln_chain(sh, L)
                for j, sl in enumerate(grp):
                    _ln_norm(b, sl, j, sh, xnth, xntl)

            def emit_ln(b, xnth, xntl):
                for g, grp in enumerate((range(0, 4), range(4, 8))):
                    emit_ln_group(b, grp, g, xnth, xntl)

            def emit_qkv_qk(h, xnth, xntl):
                """expb tile, qT/kT (fp8) for head h; 3-term fp8 DoubleRow
                (xh@wh + xl@wh + xh@wl).  v-hat is allocated here but its
                pairs are emitted inside the window interleave."""
                ones_bf, wqkh, wqkl, bqk, wvh, wvl, bv1, pw, pb1 = consts[0]
                bt = bpool.tile([128, KT * N], BF16, tag="bias")
                nc.sync.dma_start(
                    bt[:].rearrange("p (k n) -> p k n", n=N),
                    expb_d[h].rearrange("(k p) n -> p k n", p=128),
                )
                qt = qkpool.tile([64, N], E4, tag="qt")
                ktile = qkpool.tile([64, N], E4, tag="kt")
                wq_ap = lambda w, p: (
                    w[:].rearrange("p (d c) -> p d c", c=H * 128)
                    [:, 2 * p:2 * p + 2, h * 128:(h + 1) * 128])
                x_ap = lambda x, p, lo, width: (
                    x[:].rearrange("p (d n) -> p d n", n=N)
                    [:, 2 * p:2 * p + 2, lo:lo + width])
                for c in range(2):
                    qp = mpp.tile([128, 512], F32, tag="m")
                    terms = [(wqkh, xnth), (wqkh, xntl), (wqkl, xnth)]
                    for t, (w, x) in enumerate(terms):
                        for p in range(2):
                            nc.tensor.matmul(
                                qp[:], wq_ap(w, p), x_ap(x, p, c * 512, 512),
                                start=(t == 0 and p == 0),
                                stop=(not use_qk_bias and t == 2 and p == 1),
                                perf_mode=DR, skip_group_check=True,
                            )
                    if use_qk_bias:
                        nc.tensor.matmul(
                            qp[:],
                            bqk[:, h * 128:(h + 1) * 128],
                            ones_bf[:, 0:512],
                            start=False,
                            stop=True,
                        )
                    nc.vector.tensor_copy(
                        qt[:, c * 512:(c + 1) * 512], qp[0:64, :]
                    )
                    nc.vector.tensor_copy(
                        ktile[:, c * 512:(c + 1) * 512], qp[64:128, :]
                    )
                vh = vpool.tile([128, KT * VW], BF16, tag="vh")
                nc.vector.memset(
                    vh[:].rearrange("p (s w) -> p s w", w=VW)[:, :, 256:257],
                    1.0,
                )
                return (h, bt, qt, ktile, vh, xnth, xntl)

            def emit_qkv_vpair(hctx, sp):
                """One PSUM bank worth of v-hat (two tok-slices)."""
                h, bt, qt, ktile, vh, xnth, xntl = hctx
                ones_bf, wqkh, wqkl, bqk, wvh, wvl, bv1, pw, pb1 = consts[0]
                x_ap = lambda x, p, lo, width: (
                    x[:].rearrange("p (d n) -> p d n", n=N)
                    [:, 2 * p:2 * p + 2, lo:lo + width])
                wv_ap = lambda w, p: (
                    w[:].rearrange("p (d z) -> p d z", z=DH)
                    [:, 2 * p:2 * p + 2, h * 256:(h + 1) * 256])
                vp = pvpp.tile([128, 512], F32, tag="pv")
                for e in range(2):
                    sl = 2 * sp + e
                    last = (e == 1 and not use_v_bias)
                    terms = [(xnth, wvh), (xntl, wvh), (xnth, wvl)]
                    for t, (x, w) in enumerate(terms):
                        for p in range(2):
                            nc.tensor.matmul(
                                vp[:, e * 256:(e + 1) * 256],
                                x_ap(x, p, sl * 128, 128), wv_ap(w, p),
                                start=(e == 0 and t == 0 and p == 0),
                                stop=(last and t == 2 and p == 1),
                                perf_mode=DR, skip_group_check=True,
                            )
                    if use_v_bias:
                        nc.tensor.matmul(
                            vp[:, e * 256:(e + 1) * 256],
                            ones_bf[:, 0:128],
                            bv1[:, h * VW: h * VW + 256],
                            start=False,
                            stop=(e == 1),
                            skip_group_check=True,
                        )
                nc.vector.tensor_scalar(
                    vh[:].rearrange("p (s w) -> p s w", w=VW)
                       [:, 2 * sp:2 * sp + 2, 0:256],
                    vp[:].rearrange("p (two v) -> p two v", two=2),
                    1.0 / W_SCALE, None, ALU.mult,
                )

            def emit_st_kt(hctx, kt):
                """Scores DR matmul + exp + expb-mult for one k-tile of the
                *next* head.  Returns the est tile."""
                h, bt, qt, ktile, vh, xnth, xntl = hctx
                et = epool.tile([128, N], BF16, tag="e")
                for c in range(2):
                    cs = slice(c * 512, (c + 1) * 512)
                    sp = stpp.tile([128, 512], F32, tag="st")
                    nc.tensor.matmul(
                        sp[:],
                        dup2(ktile[:, kt * 128:(kt + 1) * 128]),
                        dup2(qt[:, cs]),
                        start=True, stop=True, perf_mode=DR,
                    )
                    nc.scalar.activation(
                        et[:, cs], sp[:], AF.Exp, bias=0.0, scale=EXP_SCALE
                    )
                eng = nc.gpsimd if kt in POOL_MULT_KT else nc.vector
                eng.tensor_tensor(
                    et[:], et[:], bt[:, kt * N:(kt + 1) * N], ALU.mult
                )
                return et

            def emit_pv_sl(hctx, est, sl):
                """PV matmuls + denominator + normalized attn for one
                tok-slice.  Returns the an tile."""
                h, bt, qt, ktile, vh, xnth, xntl = hctx
                pv = pvpp.tile([128, VW], F32, tag="pv")
                for kt in range(KT):
                    nc.tensor.matmul(
                        pv[:],
                        est[kt][:, sl * 128:(sl + 1) * 128],
                        vh[:, kt * VW:(kt + 1) * VW],
                        start=(kt == 0),
                        stop=(kt == KT - 1),
                    )
                rc = spool.tile([128, 1], F32, tag="rc")
                nc.vector.reciprocal(rc[:], pv[:, 256:257])
                an = apool.tile([128, 256], BF16, tag="an")
                nc.scalar.mul(an[:], pv[:, 0:256], rc[:])
                return an

            def emit_fin_sl(h, ans, sl, slab):
                """Deferred: paired transpose of normalized attn into slab."""
                tp = mpp.tile([128, 256], BF16, tag="m")
                for e in range(2):
                    nc.tensor.matmul(
                        tp[:, e * 128:(e + 1) * 128],
                        ans[sl][:, e * 128:(e + 1) * 128],
                        identb[:], is_transpose=True,
                        start=(e == 0), stop=(e == 1),
                        skip_group_check=True,
                    )
                nc.vector.tensor_copy(
                    slab[:].rearrange("p (g n) -> p g n", n=N)
                        [:, 2 * h:2 * h + 2, sl * 128:(sl + 1) * 128],
                    tp[:].rearrange("p (two n) -> p two n", two=2),
                )

            def emit_window(hctx_cur, est_cur, hctx_nxt, pend, slab, ln_q):
                """One head window: interleave scores/exp/mult of the next
                head, PV of the current head, fin of the previous, and at
                most one LN sub-task per cycle."""
                est_nxt = [] if hctx_nxt is not None else None
                ans = []
                for i in range(KT):
                    if hctx_nxt is not None:
                        est_nxt.append(emit_st_kt(hctx_nxt, i))
                        if i % 2 == 1:
                            emit_qkv_vpair(hctx_nxt, i // 2)
                    ans.append(emit_pv_sl(hctx_cur, est_cur, i))
                    if pend is not None:
                        emit_fin_sl(pend[0], pend[1], i, slab)
                    if ln_q:
                        ln_q.pop(0)()
                return est_nxt, ans

            def emit_proj(b, slab, pend):
                ones_bf, wqkh, wqkl, bqk, wvh, wvl, bv1, pw, pb1 = consts[0]
                for sl in range(QS):
                    if pend is not None:
                        emit_fin_sl(pend[0], pend[1], sl, slab)
                    pp = stpp.tile([128, DIM], F32, tag="st")
                    for dh in range(16):
                        nc.tensor.matmul(
                            pp[:],
                            slab[:, dh * N + sl * 128: dh * N + (sl + 1) * 128],
                            pw[:, dh * DIM:(dh + 1) * DIM],
                            start=(dh == 0),
                            stop=(not use_pb and dh == 15),
                        )
                    if use_pb:
                        nc.tensor.matmul(
                            pp[:], ones_bf[:, 0:128], pb1[:], start=False,
                            stop=True, skip_group_check=True,
                        )
                    yt = ypool.tile([128, DIM], F32, tag="y")
                    nc.vector.tensor_copy(yt[:], pp[:])
                    nc.sync.dma_start(y_d[b, sl * 128:(sl + 1) * 128, :], yt[:])

            # ---- software-pipelined main loop --------------------------
            xnt_cur = (xpool.tile([128, DT * N], E4, tag="xnth", name="xnth1"),
                       xpool.tile([128, DT * N], E5, tag="xntl", name="xntl1"))
            emit_ln(0, *xnt_cur)
            consts[0] = load_consts()
            hctx_cur = emit_qkv_qk(0, *xnt_cur)
            load_pw()
            for sp in range(QS // 2):
                emit_qkv_vpair(hctx_cur, sp)
            est_cur = [emit_st_kt(hctx_cur, kt) for kt in range(KT)]
            slab = slabpool.tile([128, 16 * N], BF16, tag="slab")
            LN_AT = {4: range(0, 3), 5: range(3, 6), 6: range(6, 8)}
            xnt_next = None
            pend = None
            ln_q = []
            for b in range(BL):
                for h in range(H):
                    if h + 1 < H:
                        hctx_nxt = emit_qkv_qk(h + 1, *xnt_cur)
                        if b + 1 < BL and h in LN_AT:
                            if h == 4:
                                xnt_next = (
                                    xpool.tile([128, DT * N], E4,
                                               tag="xnth", name="xnth2"),
                                    xpool.tile([128, DT * N], E5,
                                               tag="xntl", name="xntl2"),
                                )
                            emit_ln_group(b + 1, LN_AT[h], h, *xnt_next)
                    elif b + 1 < BL:
                        hctx_nxt = emit_qkv_qk(0, *xnt_next)
                    else:
                        hctx_nxt = None
                    est_nxt, ans = emit_window(
                        hctx_cur, est_cur, hctx_nxt, pend, slab, ln_q
                    )
                    pend = (h, ans)
                    hctx_cur, est_cur = hctx_nxt, est_nxt
                emit_proj(b, slab, pend)
                pend = None
                xnt_cur = xnt_next

    nc.compile()
    return nc


_CACHE = {}


def _prep_host(gamma, beta, qkv_w, qkv_b, proj_w, proj_b, biases, bias_idxs):
    import ml_dtypes

    qkv_w = np.asarray(qkv_w, np.float32)
    qkv_b = np.asarray(qkv_b, np.float32)
    gamma = np.asarray(gamma, np.float32)
    beta = np.asarray(beta, np.float32)
    w = qkv_w * gamma[:, None]          # fold LN gamma
    bfold = qkv_b + beta @ qkv_w        # fold LN beta
    w3 = w.reshape(DIM, H, 384)
    b3 = bfold.reshape(H, 384)
    # q/k columns scaled x(16*QK_PRE); v columns x16; exp scale / the 1/16
    # copy-out divide it back.  Weights split hi (e4m3) + lo (e5m2).
    wqk = (w3[:, :, :128] * (QK_PRE * W_SCALE)).reshape(DIM, H * 128)
    bqk = (b3[:, :128] * (QK_PRE * W_SCALE)).reshape(1, H * 128)
    wv = (w3[:, :, 128:] * W_SCALE).reshape(DIM, DH)
    bv = b3[:, 128:] * W_SCALE          # [H, 256]
    bv1 = np.concatenate(
        [bv, np.ones((H, 1), np.float32)], axis=1,
    ).reshape(1, H * VW)
    wqkh = wqk.astype(ml_dtypes.float8_e4m3)
    wqkl = (wqk - wqkh.astype(np.float32)).astype(ml_dtypes.float8_e5m2)
    wvh = wv.astype(ml_dtypes.float8_e4m3)
    wvl = (wv - wvh.astype(np.float32)).astype(ml_dtypes.float8_e5m2)
    bias_full = np.asarray(biases, np.float32)[:, np.asarray(bias_idxs)]
    # device multiplies est[k, q] by exp(bias[q, k])^T
    expb = np.exp(bias_full.transpose(0, 2, 1))
    return {
        "wqkh": wqkh,
        "wqkl": wqkl,
        "wvh": wvh,
        "wvl": wvl,
        "bqk": bqk.astype(ml_dtypes.bfloat16),
        "bv1": bv1.astype(ml_dtypes.bfloat16),
        "pw": np.ascontiguousarray(np.asarray(proj_w, np.float32)).astype(ml_dtypes.bfloat16),
        "pb1": np.asarray(proj_b, np.float32).reshape(1, DIM).astype(ml_dtypes.bfloat16),
        "expb": np.ascontiguousarray(expb).astype(ml_dtypes.bfloat16),
        "identb": np.eye(128, dtype=np.float32).astype(ml_dtypes.bfloat16),
        "ones": np.ones((1, 512), ml_dtypes.bfloat16),
    }


def kernel(x, gamma, beta, qkv_w, qkv_b, proj_w, proj_b, biases, bias_idxs,
           _trace=False, _tmpdir=None):
    x = np.asarray(x, np.float32)
    shared = _prep_host(gamma, beta, qkv_w, qkv_b, proj_w, proj_b, biases,
                        bias_idxs)
    flags = (
        bool(np.any(np.asarray(shared["bqk"], np.float32))),
        bool(np.any(np.asarray(shared["bv1"], np.float32)
                    .reshape(H, VW)[:, :256])),
        bool(np.any(np.asarray(shared["pb1"], np.float32))),
    )
    if _CACHE.get("flags") != flags:
        _CACHE["nc"] = build_program(*flags)
        _CACHE["flags"] = flags
    nc = _CACHE["nc"]
    in_maps = []
    for c in range(NCORES):
        m = dict(shared)
        m["x"] = np.ascontiguousarray(x[c * BL:(c + 1) * BL])
        in_maps.append(m)
    res = run_bass_kernel_spmd(
        nc, in_maps, list(range(NCORES)), trace=_trace, tmpdir=_tmpdir,
    )
    _CACHE["last"] = res
    out = np.concatenate([res.results[c]["y"] for c in range(NCORES)], axis=0)
    return out.astype(np.float32)
